# revision 7
# baseline (speedup 1.0000x reference)
"""Distributed Trainium2 kernel for nn_Attention_21990232555717.

Reference (per batch element a, seq s=1024, model dim c=1024, 16 heads):
    qkv = x @ w_qkv                       # (s, 3072)
    q,k,v split per head (hd=64)
    scores = q @ k.T * (1/sqrt(1024))     # (h, s, s)
    attn = softmax(scores, axis=HEADS)    # normalize across the 16 heads!
    out = attn @ v -> (s, 1024) @ w_out + b_out

Sharding: pure data parallel - batch (8) across 8 cores, weights replicated.

v2 design (vs the phase-sequential baseline):
  * all-bf16 datapath (rel err ~5e-3 vs the 2e-2 gate; verified in numpy)
  * weights arrive as bf16 via SWDGE cast-DMA (no ACT/DVE cast work)
  * attention starts DURING the projections: qk_proj is emitted per
    head-pair (K_p then Q_p) and scores+exp for qb0 chase the pairs, so
    the ACT exp stream (the ~147us serial bottleneck: 128 exps of
    [128,1024]) begins ~30us in instead of ~110us.
  * engine roles: ACT = exps + QKT psum copies (+qb0 spills);
    DVE = den chains (heads 8-15) + recip + normalize muls + merges +
    y bias; GpSimd = den partial sums over heads 0-7 (own queue,
    off critical path); SWDGE = all weight casts.
  * PSUM: one [128,1024] pool (bufs=2, 4 banks) shared by transposes /
    projections / scores; one [128,512] pool (bufs=4, 4 banks) for
    attnV accumulators + out-proj units (spill-half attnV structure).
  * SBUF aliasing: the dead wqk slot after qk_proj is re-viewed as the
    3rd E-group buffer; the dead wv slot after v_proj hosts outT and
    the attnV spill partials.
"""

import numpy as np

import concourse.bass as bass
import concourse.mybir as mybir
import concourse.tile as tile
from concourse import bacc
from concourse.bass import broadcast_tensor_aps
from concourse.bass_utils import run_bass_kernel_spmd
from concourse.masks import make_identity

F32 = mybir.dt.float32
BF16 = mybir.dt.bfloat16
Exp = mybir.ActivationFunctionType.Exp
Bypass = mybir.AluOpType.bypass
Add = mybir.AluOpType.add

S = 1024      # sequence length per core (batch element)
C = 1024      # model dim
H = 16        # heads
HD = 64       # head dim
SCALE = 1.0 / (C ** 0.5)
QB = 256      # q block size
NQB = S // QB          # 4 q blocks
NKT = S // 128         # 8 k tiles
NCT = C // 128         # 8 contraction tiles
NPAIR = 8              # head pairs


def build():
    nc = bacc.Bacc(None, target_bir_lowering=False)
    x_ext = nc.declare_dram_parameter("x", [S, C], F32, isOutput=False)
    wqkv_ext = nc.declare_dram_parameter("w_qkv", [C, 3 * C], F32, isOutput=False)
    wout_ext = nc.declare_dram_parameter("w_out", [C, C], F32, isOutput=False)
    b_ext = nc.declare_dram_parameter("b_out", [C], F32, isOutput=False)
    out_ext = nc.declare_dram_parameter("out", [S, C], F32, isOutput=True)

    with tile.TileContext(nc) as tc:
        with (
            tc.tile_pool(name="const_p", bufs=1) as const_p,
            tc.tile_pool(name="xf_p", bufs=1) as xf_p,
            tc.tile_pool(name="xb_p", bufs=1) as xb_p,
            tc.tile_pool(name="xt_p", bufs=1) as xt_p,      # xT, then wout
            tc.tile_pool(name="w_p", bufs=1) as w_p,        # wqk->E3, wv->scratch
            tc.tile_pool(name="act_p", bufs=1) as act_p,
            tc.tile_pool(name="e_p", bufs=2) as e_p,
            tc.tile_pool(name="d_p", bufs=1) as d_p,
            tc.tile_pool(name="r_p", bufs=1) as r_p,
            tc.tile_pool(name="y_p", bufs=2) as y_p,
            tc.tile_pool(name="ps_w", bufs=2, space="PSUM") as ps_w,
            tc.tile_pool(name="ps_a", bufs=4, space="PSUM") as ps_a,
        ):
            # ---- constants ----
            ident = const_p.tile([128, 128], BF16)
            make_identity(nc, ident)
            ones1 = const_p.tile([1, 128], BF16)
            nc.vector.memset(ones1, 1.0)
            b_sb = const_p.tile([1, C], BF16)
            nc.gpsimd.dma_start(b_sb, b_ext[None, :])

            # ---- persistent activations ----
            QKT = act_p.tile([128, H, S], BF16)        # 32 KB/part (ft 0-7 Q, 8-15 K)
            Vb = act_p.tile([128, NKT, C], BF16)       # 16 KB/part
            b_bcast = act_p.tile([128, C], BF16)       # 2 KB/part

            # ---- weights (SWDGE cast-DMA f32 -> bf16) ----
            wqk = w_p.tile([128, NCT, 2 * C], BF16, tag="wqk", name="wqk")  # 32 KB
            wv = w_p.tile([128, NCT, C], BF16, tag="wv", name="wv")         # 16 KB
            for ct in range(NCT):
                nc.gpsimd.dma_start(wqk[:, ct, :],
                                    wqkv_ext[ct * 128:(ct + 1) * 128, 0:2 * C])
            for ct in range(NCT):
                nc.gpsimd.dma_start(wv[:, ct, :],
                                    wqkv_ext[ct * 128:(ct + 1) * 128, 2 * C:3 * C])

            # ---- x: HWDGE f32 + DVE cast + PE transpose (bf16) ----
            xT = xt_p.tile([128, NCT, S], BF16, tag="xt", name="xT")  # 16 KB
            for st in range(NKT):
                xf = xf_p.tile([128, C], F32, tag="xf", name=f"xf{st}")
                eng = nc.sync if st % 2 == 0 else nc.scalar
                eng.dma_start(xf, x_ext[st * 128:(st + 1) * 128, :])
                xb = xb_p.tile([128, C], BF16, tag="xb", name=f"xb{st}")
                nc.vector.tensor_copy(xb, xf)
                pt = ps_w.tile([128, S], F32, tag="pw", name=f"pt{st}")
                ptb = pt.bitcast(BF16)  # bf16 view: transpose out must be bf16
                for ct in range(NCT):
                    nc.tensor.transpose(ptb[:, ct * 128:(ct + 1) * 128],
                                        xb[:, ct * 128:(ct + 1) * 128], ident)
                ptv = ptb[:, 0:C].rearrange("p (a b) -> p a b", a=NCT)
                nc.vector.tensor_copy(xT[:, :, st * 128:(st + 1) * 128], ptv)

            # b_bcast: broadcast bias to all partitions via ones-matmul
            for ec in range(2):
                psb = ps_a.tile([128, 512], F32, tag="acc", name=f"psbb{ec}")
                nc.tensor.matmul(psb, ones1, b_sb[:, ec * 512:(ec + 1) * 512],
                                 start=True, stop=True)
                nc.vector.tensor_copy(b_bcast[:, ec * 512:(ec + 1) * 512], psb)

            # ================= interleaved main pipeline =================
            Egrp = {}      # (qb, gg) -> E tile view [128, H, 4*QB]
            SLOW = {}      # (qb, gg) -> gpsimd partial sums (heads 0-7)
            UD = {}        # (qb, gg) -> dve chains (heads 8-15)
            recg = {}      # (qb, gg) -> rec bf16 [128, 4*QB]

            def emit_qk_ft(ft):
                ps = ps_w.tile([128, S], F32, tag="pw", name=f"qk{ft}")
                for ct in range(NCT):
                    lhsT = wqk[:, ct, ft * 128:(ft + 1) * 128]
                    for sb in range(2):
                        nc.tensor.matmul(
                            ps[:, sb * 512:(sb + 1) * 512], lhsT,
                            xT[:, ct, sb * 512:(sb + 1) * 512],
                            start=(ct == 0), stop=(ct == NCT - 1))
                nc.scalar.copy(QKT[:, ft, :], ps)

            def emit_v_st(st):
                ps = ps_w.tile([128, S], F32, tag="pw", name=f"v{st}")
                for ct in range(NCT):
                    lhsT = xT[:, ct, st * 128:(st + 1) * 128]
                    for fb in range(2):
                        nc.tensor.matmul(
                            ps[:, fb * 512:(fb + 1) * 512], lhsT,
                            wv[:, ct, fb * 512:(fb + 1) * 512],
                            start=(ct == 0), stop=(ct == NCT - 1))
                nc.vector.tensor_copy(Vb[:, st, :], ps)

            def new_group(qb, gg):
                i = 2 * qb + gg
                if i % 3 == 2:
                    # 3rd ring slot: re-view the dead wqk slot (same bytes)
                    ew = w_p.tile([128, NCT, 2 * C], BF16, tag="wqk",
                                  name=f"Ew{i}")
                    Egrp[(qb, gg)] = ew.rearrange("p a (b c) -> p (a b) c",
                                                  b=2, c=1024)
                else:
                    Egrp[(qb, gg)] = e_p.tile([128, H, 4 * QB], BF16, tag="E",
                                              name=f"E{qb}_{gg}")
                SLOW[(qb, gg)] = d_p.tile([128, 2, 4 * QB], BF16, tag="slow",
                                          name=f"sl{qb}_{gg}")
                UD[(qb, gg)] = d_p.tile([128, 2, 4 * QB], BF16, tag="ud",
                                        name=f"ud{qb}_{gg}")

            def emit_scores(qb, gg, h):
                """scores + exp + gpsimd denominator links for head h.

                Denominator: heads 0-7 summed on GpSimd (own queue, chases
                the exps, off critical path); heads 8-15 summed on DVE via
                emit_chains_dve (emitted separately so the in-order DVE
                queue never parks next-group chain adds ahead of this
                group's tail/muls)."""
                E = Egrp[(qb, gg)]
                sl = SLOW[(qb, gg)]
                q0 = qb * QB
                po = 64 * (h % 2)
                rhs = QKT[po:po + 64, h // 2, q0:q0 + QB]
                pss = ps_w.tile([128, 4 * QB], F32, tag="pw",
                                name=f"sc{qb}_{gg}_{h}")
                for j in range(4):
                    kt = 4 * gg + j
                    lhsT = QKT[po:po + 64, 8 + h // 2, kt * 128:(kt + 1) * 128]
                    nc.tensor.matmul(pss[:, j * QB:(j + 1) * QB], lhsT, rhs,
                                     start=True, stop=True)
                nc.scalar.activation(E[:, h, :], pss, Exp, scale=SCALE)
                # gpsimd tree over heads 0-7:
                #   sl0 = E0+E1 (+E2) (+E3); sl1 = E4+E5 (+E6) (+E7); sl0 += sl1
                if h < 8 and h % 2 == 1:
                    c = h // 4           # chain 0 for heads 0-3, 1 for 4-7
                    if h % 4 == 1:
                        nc.gpsimd.tensor_add(sl[:, c, :], E[:, h - 1, :],
                                             E[:, h, :])
                    else:
                        nc.gpsimd.tensor_add(sl[:, c, :], sl[:, c, :],
                                             E[:, h - 1, :])
                        nc.gpsimd.tensor_add(sl[:, c, :], sl[:, c, :],
                                             E[:, h, :])
                        if h == 7:
                            nc.gpsimd.tensor_add(sl[:, 0, :], sl[:, 0, :],
                                                 sl[:, 1, :])

            def emit_chains_dve(qb, gg):
                """DVE pairwise sums over heads 8-15 of (qb, gg)."""
                E = Egrp[(qb, gg)]
                ud = UD[(qb, gg)]
                for c in range(2):
                    h0 = 8 + 4 * c
                    nc.vector.tensor_add(ud[:, c, :], E[:, h0, :], E[:, h0 + 1, :])
                    nc.vector.tensor_add(ud[:, c, :], ud[:, c, :], E[:, h0 + 2, :])
                    nc.vector.tensor_add(ud[:, c, :], ud[:, c, :], E[:, h0 + 3, :])
                nc.vector.tensor_add(ud[:, 0, :], ud[:, 0, :], ud[:, 1, :])

            def emit_den_tail_and_muls(qb, gg):
                E = Egrp[(qb, gg)]
                sl = SLOW[(qb, gg)]
                ud = UD[(qb, gg)]
                denf = r_p.tile([128, 4 * QB], F32, tag="denf",
                                name=f"denf{qb}_{gg}")
                rec = r_p.tile([128, 4 * QB], BF16, tag="rec",
                               name=f"rec{qb}_{gg}")
                recg[(qb, gg)] = rec
                nc.vector.tensor_add(denf, ud[:, 0, :], sl[:, 0, :])
                nc.vector.reciprocal_approx_fast(out=denf, in_=denf)
                nc.vector.tensor_copy(rec, denf)
                # normalize: wave-ordered pair muls so attnV can chase
                for w in range(NPAIR):
                    esl = E[:, 2 * w:2 * w + 2, :]
                    rb, _ = broadcast_tensor_aps(rec[:, None, :], esl)
                    nc.vector.tensor_mul(esl, esl, rb)

            def emit_attnv_wave(qb, w, half, outT, partials):
                """attnV for head pair w over k-tiles of half (4 kt).
                half 0 spills to partials; half 1 merges into outT."""
                aw = ps_a.tile([128, 512], F32, tag="acc",
                               name=f"aw{qb}_{half}_{w}")
                E = Egrp[(qb, half)]
                kts = range(4 * half, 4 * half + 4)
                for kt in kts:
                    j = kt % 4
                    for i in range(2):
                        h = 2 * w + i
                        po = 64 * (h % 2)
                        nc.tensor.matmul(
                            aw[po:po + 64, 0:QB],
                            Vb[:, kt, h * HD:(h + 1) * HD],
                            E[:, h, j * QB:(j + 1) * QB],
                            start=(kt == kts[0]), stop=(kt == kts[-1]),
                            tile_position=(0, po))
                if half == 0:
                    if qb == 0:
                        nc.scalar.copy(partials[:, w, :], aw[:, 0:QB])
                    else:
                        nc.vector.tensor_copy(partials[:, w, :], aw[:, 0:QB])
                else:
                    nc.vector.scalar_tensor_tensor(
                        outT[:, w, :], partials[:, w, :], 0.0,
                        aw[:, 0:QB], Bypass, Add)

            def emit_out_proj_qsub(qb, outT, qsub, wout):
                q0 = qb * QB
                psy = [ps_a.tile([128, 512], F32, tag="acc",
                                 name=f"psy{qb}_{qsub}_{ec}") for ec in range(2)]
                for ft in range(NCT):
                    lhsT = outT[:, ft, qsub * 128:(qsub + 1) * 128]
                    for ec in range(2):
                        nc.tensor.matmul(psy[ec], lhsT,
                                         wout[:, ft, ec * 512:(ec + 1) * 512],
                                         start=(ft == 0), stop=(ft == NCT - 1))
                for ec in range(2):
                    y = y_p.tile([128, 512], F32, tag="y",
                                 name=f"y{qb}_{qsub}_{ec}")
                    nc.vector.scalar_tensor_tensor(
                        y, psy[ec], 0.0, b_bcast[:, ec * 512:(ec + 1) * 512],
                        Bypass, Add)
                    nc.sync.dma_start(
                        out_ext[q0 + qsub * 128:q0 + (qsub + 1) * 128,
                                ec * 512:(ec + 1) * 512], y)

            # ---------- phase A: qk pairs + v tiles + qb0 scores ----------
            new_group(0, 0)
            new_group(0, 1)
            # v_st tiles interleaved after pair 2 (wv cast-DMA lands ~30us)
            vsched = {3: [0], 4: [1, 2], 5: [3, 4], 6: [5, 6], 7: [7]}
            with nc.named_scope("proj_attn_overlap"):
                for p in range(NPAIR):
                    emit_qk_ft(8 + p)   # K pair p
                    emit_qk_ft(p)       # Q pair p
                    for st in vsched.get(p, []):
                        emit_v_st(st)
                    # scores for the previous pair (1-pair lag keeps PE fed)
                    if p >= 1:
                        for gg in range(2):
                            for i in range(2):
                                emit_scores(0, gg, 2 * (p - 1) + i)
                for gg in range(2):
                    for i in range(2):
                        emit_scores(0, gg, 2 * 7 + i)
                emit_chains_dve(0, 0)
                emit_chains_dve(0, 1)

            # wout: reuses the xT pool slot (xT dead after v_proj)
            wout = xt_p.tile([128, NCT, C], BF16, tag="xt", name="wout")
            for ct in range(NCT):
                nc.gpsimd.dma_start(wout[:, ct, :],
                                    wout_ext[ct * 128:(ct + 1) * 128, :])

            # scratch: reuses the wv slot (dead after v_proj) for outT (even/
            # odd qb) and the attnV spill partials - disjoint column ranges,
            # range-level deps keep reuse across qbs correct.
            scratch = w_p.tile([128, NCT, C], BF16, tag="wv", name="scratch")
            outT_views = [scratch[:, :, 0:QB], scratch[:, :, QB:2 * QB]]
            partials = scratch[:, :, 2 * QB:3 * QB]

            # ---------- phase B: attention pipeline over qbs ----------
            for qb in range(NQB):
                with nc.named_scope(f"attn_qb{qb}"):
                    outT = outT_views[qb % 2]
                    # next-group scores first: ACT can run ahead into
                    # (qb+1, 0) while this qb's muls/attnV are pending
                    if qb + 1 < NQB:
                        new_group(qb + 1, 0)
                        for h in range(H):
                            emit_scores(qb + 1, 0, h)
                    emit_den_tail_and_muls(qb, 0)
                    for w in range(NPAIR):
                        emit_attnv_wave(qb, w, 0, outT, partials)
                    if qb + 1 < NQB:
                        new_group(qb + 1, 1)
                        for h in range(H):
                            emit_scores(qb + 1, 1, h)
                    emit_den_tail_and_muls(qb, 1)
                    for w in range(NPAIR):
                        emit_attnv_wave(qb, w, 1, outT, partials)
                    emit_out_proj_qsub(qb, outT, 0, wout)
                    emit_out_proj_qsub(qb, outT, 1, wout)
                    # DVE chain adds for the next qb's groups go LAST in the
                    # DVE queue (they wait on that qb's exps; anything queued
                    # behind them would stall ~18us)
                    if qb + 1 < NQB:
                        emit_chains_dve(qb + 1, 0)
                        emit_chains_dve(qb + 1, 1)

    nc.compile()
    return nc


_NC = None


def _get_nc():
    global _NC
    if _NC is None:
        _NC = build()
    return _NC


def kernel(x, w_qkv, w_out, b_out):
    nc = _get_nc()
    x = np.ascontiguousarray(np.asarray(x, dtype=np.float32))
    w_qkv = np.ascontiguousarray(np.asarray(w_qkv, dtype=np.float32))
    w_out = np.ascontiguousarray(np.asarray(w_out, dtype=np.float32))
    b_out = np.ascontiguousarray(np.asarray(b_out, dtype=np.float32))
    in_maps = [
        {"x": x[i], "w_qkv": w_qkv, "w_out": w_out, "b_out": b_out}
        for i in range(8)
    ]
    res = run_bass_kernel_spmd(nc, in_maps, core_ids=list(range(8)))
    out = np.stack([np.asarray(res.results[i]["out"]) for i in range(8)])
    return out.astype(np.float32)
